# revision 1
# baseline (speedup 1.0000x reference)
"""Expert-choice MoE router on 8 Trainium2 NeuronCores.

Sharding: data-parallel over the batch dim (B=8 rows -> 8 cores). Each core
computes its row's full MLP router (Linear(4096,1024) -> exact GELU ->
Linear(1024,1) -> sigmoid) plus the per-row variable-k top-k selection.

The big matmul runs as a 3-pass fp16 hi/lo split (hi*hi + hi*lo + lo*hi) on
the PE array: fp16 products are exact in fp32 accumulation, so the dropped
lo*lo term (~2^-22 relative) keeps the logit error ~1e-7 — far below the
~5e-5 minimum top-k boundary gap — while running at the full 1-cycle/row PE
rate instead of fp32's 4 cycles/row.

Top-k uses a fixed 26-step threshold bisection on masked logits (selection by
logit order == selection by sigmoid-score order): count(logit > t) computed on
DVE with a 32x32-transpose partition reduction, no host round trips.
"""
import numpy as np

import bass_rust
import concourse.bass as bass
import concourse.mybir as mybir
import concourse.tile as tile
from concourse.bass_utils import run_bass_kernel_spmd

B, S, D, H = 8, 4096, 4096, 1024
KC = D // 128          # 32 contraction chunks
HC = H // 128          # 8 hidden chunks
TT = 512               # token tile (free dim of mm1)
NT = S // TT           # 8 token tiles
N_ITER = 26            # bisection steps: 16 * 2^-26 = 2.4e-7 interval
LOGIT_BOUND = 8.0

F32 = mybir.dt.float32
F16 = mybir.dt.float16
U8 = mybir.dt.uint8
I32 = mybir.dt.int32
AF = mybir.ActivationFunctionType
ALU = mybir.AluOpType


def _install_drain_split_patch():
    """The installed walrus build accepts fewer sync waits per instruction
    than bass/Tile emits; split multi-wait instructions into single-wait NOPs."""
    if getattr(tile.TileContext, "_drain_split_patched", False):
        return

    def split_multi_waits(nc, max_waits=1):
        ctr = 0
        for fn in nc.m.functions:
            for blk in fn.blocks:
                new = []
                changed = False
                for inst in blk.instructions:
                    si = inst.sync_info
                    waits = list(si.on_wait) if si is not None and si.on_wait else []
                    if len(waits) > max_waits:
                        for w in waits[:-max_waits]:
                            ctr += 1
                            new.append(mybir.InstNoOp(
                                name=f"WS-{ctr}",
                                engine=inst.engine,
                                sync_info=mybir.SyncInfo(on_wait=[w], on_update=[]),
                                bass_nofuse=True,
                            ))
                        si.on_wait = waits[-max_waits:]
                        changed = True
                    new.append(inst)
                if changed:
                    blk.instructions = new

    orig = tile.TileContext._drain_and_barrier

    def patched(self, tick_clock, wait_clock):
        orig(self, tick_clock, wait_clock)
        split_multi_waits(self.nc)

    tile.TileContext._drain_and_barrier = patched
    tile.TileContext._drain_split_patched = True


def build_program(passes=3, do_mm2=True, do_tail=True, n_iter=N_ITER):
    _install_drain_split_patch()
    nc = bass.Bass()

    hs_t = nc.dram_tensor("hs_t", [D, S], F32, kind="ExternalInput")
    w1hi = nc.dram_tensor("w1hi", [D, H], F16, kind="ExternalInput")
    w1lo = nc.dram_tensor("w1lo", [D, H], F16, kind="ExternalInput")
    b1pk = nc.dram_tensor("b1pk", [128, HC], F32, kind="ExternalInput")
    w2pk = nc.dram_tensor("w2pk", [128, HC], F32, kind="ExternalInput")
    b2rep = nc.dram_tensor("b2rep", [32, 1], F32, kind="ExternalInput")
    am_t = nc.dram_tensor("am_t", [32, 128], U8, kind="ExternalInput")

    o_rw = nc.dram_tensor("o_rw", [S], F32, kind="ExternalOutput")
    o_sel = nc.dram_tensor("o_sel", [S], U8, kind="ExternalOutput")
    logit_dram = nc.dram_tensor("logit_scratch", [S], F32, kind="Internal")

    with tile.TileContext(nc) as tc:
        with (
            tc.tile_pool(name="wres", bufs=1) as wres,
            tc.tile_pool(name="xin", bufs=3) as xin,
            tc.tile_pool(name="hact", bufs=2) as hpool,
            tc.tile_pool(name="ps", bufs=8, space="PSUM") as ps,
            tc.tile_pool(name="tail", bufs=2) as tp,
        ):
            # resident weights
            w1hi_sb = wres.tile([128, KC, H], F16)
            nc.sync.dma_start(w1hi_sb[:], w1hi.rearrange("(k p) h -> p k h", p=128))
            w1lo_sb = wres.tile([128, KC, H], F16)
            nc.sync.dma_start(w1lo_sb[:], w1lo.rearrange("(k p) h -> p k h", p=128))
            b1_sb = wres.tile([128, HC], F32)
            nc.sync.dma_start(b1_sb[:], b1pk[:])
            w2_sb = wres.tile([128, HC], F32)
            nc.sync.dma_start(w2_sb[:], w2pk[:])
            b2_sb = wres.tile([32, 1], F32)
            nc.sync.dma_start(b2_sb[:], b2rep[:])
            am_sb = wres.tile([32, 128], U8)
            nc.sync.dma_start(am_sb[:], am_t[:])

            logits128 = wres.tile([128, NT * 4], F32)

            hs_v = hs_t.rearrange("(k p) t -> k p t", p=128)

            for T in range(NT):
                psum = [ps.tile([128, TT], F32, tag="ps", name=f"psum{T}_{h}")
                        for h in range(HC)]
                for k in range(KC):
                    xf = xin.tile([128, TT], F32, tag="xf")
                    nc.sync.dma_start(xf[:], hs_v[k, :, T * TT:(T + 1) * TT])
                    xhi = xin.tile([128, TT], F16, tag="xhi")
                    nc.scalar.copy(xhi[:], xf[:])
                    xlo = xin.tile([128, TT], F16, tag="xlo")
                    nc.vector.tensor_sub(xlo[:], xf[:], xhi[:])
                    for h in range(HC):
                        whi = w1hi_sb[:, k, h * 128:(h + 1) * 128]
                        wlo = w1lo_sb[:, k, h * 128:(h + 1) * 128]
                        nc.tensor.matmul(psum[h][:], whi, xhi[:],
                                         start=(k == 0), stop=(passes == 1 and k == KC - 1))
                        if passes >= 2:
                            nc.tensor.matmul(psum[h][:], whi, xlo[:],
                                             start=False, stop=(passes == 2 and k == KC - 1))
                        if passes >= 3:
                            nc.tensor.matmul(psum[h][:], wlo, xhi[:],
                                             start=False, stop=(k == KC - 1))
                hacts = []
                for h in range(HC):
                    ha = hpool.tile([128, TT], F32, tag=f"ha{h}")
                    nc.scalar.activation(ha[:], psum[h][:], AF.Gelu,
                                         bias=b1_sb[:, h:h + 1])
                    hacts.append(ha)
                if do_mm2:
                    ps2 = ps.tile([128, 4], F32, tag="ps")
                    for c in range(4):
                        for h in range(HC):
                            nc.tensor.matmul(
                                ps2[:, c:c + 1],
                                hacts[h][:, c * 128:(c + 1) * 128],
                                w2_sb[:, h:h + 1],
                                start=(h == 0), stop=(h == HC - 1))
                    nc.vector.tensor_copy(logits128[:, T * 4:(T + 1) * 4], ps2[:])
                else:
                    nc.vector.tensor_copy(logits128[:, T * 4:(T + 1) * 4],
                                          hacts[0][:, 0:4])

            if not do_tail:
                nc.sync.dma_start(o_rw.rearrange("(j p) -> p j", p=128), logits128[:])
                o_sel_v = o_sel.rearrange("(f q) -> q f", q=32)
                nc.sync.dma_start(o_sel_v, am_sb[:])
                return nc

            # reorder [128, 32] (token = col*128 + p) -> [32, 128] (token = col*32 + p)
            nc.sync.dma_start(logit_dram.rearrange("(j p) -> p j", p=128), logits128[:])
            lg = tp.tile([32, 128], F32, tag="lg")
            nc.sync.dma_start(lg[:], logit_dram.rearrange("(f q) -> q f", q=32))

            # masked logits
            mkf = tp.tile([32, 128], F32, tag="mkf")
            nc.vector.tensor_copy(mkf[:], am_sb[:])
            negbig = tp.tile([32, 128], F32, tag="negbig")
            nc.vector.memset(negbig[:], -1.0e30)
            ml = tp.tile([32, 128], F32, tag="ml")
            nc.vector.select(ml[:], am_sb[:], lg[:], negbig[:])

            # num_active (replicated across the 32 partitions)
            def preduce(src128):  # [32, 1] partials -> [32, 1] replicated total
                bc = tp.tile([32, 32], F32, tag="bc")
                nc.vector.tensor_copy(bc[:], src128.broadcast_to([32, 32]))
                tr = tp.tile([32, 32], F32, tag="tr")
                nc.vector.transpose(tr[:], bc[:])
                tot = tp.tile([32, 1], F32, tag="tot")
                nc.vector.reduce_sum(tot[:], tr[:], axis=mybir.AxisListType.X)
                return tot

            pc_na = tp.tile([32, 1], F32, tag="pc")
            nc.vector.reduce_sum(pc_na[:], mkf[:], axis=mybir.AxisListType.X)
            na = preduce(pc_na[:])

            # k = min(max(rne(0.5*na - 0.25), 1), na)   (== clamped floor)
            k0f = tp.tile([32, 1], F32, tag="k0f")
            nc.vector.tensor_scalar(k0f[:], na[:], 0.5, -0.25,
                                    op0=ALU.mult, op1=ALU.add)
            k0i = tp.tile([32, 1], I32, tag="k0i")
            nc.vector.tensor_copy(k0i[:], k0f[:])
            kf = tp.tile([32, 1], F32, tag="kf")
            nc.vector.tensor_copy(kf[:], k0i[:])
            nc.vector.tensor_scalar_max(kf[:], kf[:], 1.0)
            krep = tp.tile([32, 1], F32, tag="krep")
            nc.vector.tensor_tensor(krep[:], kf[:], na[:], op=ALU.min)

            # threshold bisection on masked logits
            lo = tp.tile([32, 1], F32, tag="lo")
            nc.vector.memset(lo[:], -LOGIT_BOUND)
            hi = tp.tile([32, 1], F32, tag="hi")
            nc.vector.memset(hi[:], LOGIT_BOUND)
            ge_scr = tp.tile([32, 128], F32, tag="ge_scr")
            for it in range(n_iter):
                mid = tp.tile([32, 1], F32, tag="mid")
                nc.vector.tensor_tensor(mid[:], lo[:], hi[:], op=ALU.add)
                nc.vector.tensor_scalar_mul(mid[:], mid[:], 0.5)
                pc = tp.tile([32, 1], F32, tag="pc")
                nc.vector.tensor_scalar(ge_scr[:], ml[:], mid[:], None,
                                        op0=ALU.is_gt)
                nc.vector.reduce_sum(pc[:], ge_scr[:], axis=mybir.AxisListType.X)
                cnt = preduce(pc[:])
                gek = tp.tile([32, 1], U8, tag="gek")
                nc.vector.tensor_tensor(gek[:], cnt[:], krep[:], op=ALU.is_ge)
                nlo = tp.tile([32, 1], F32, tag="lo")
                nc.vector.select(nlo[:], gek[:], mid[:], lo[:])
                nhi = tp.tile([32, 1], F32, tag="hi")
                nc.vector.select(nhi[:], gek[:], hi[:], mid[:])
                lo, hi = nlo, nhi

            sel = tp.tile([32, 128], F32, tag="sel")
            nc.vector.tensor_scalar(sel[:], ml[:], lo[:], None, op0=ALU.is_gt)

            scores = tp.tile([32, 128], F32, tag="scores")
            nc.scalar.activation(scores[:], lg[:], AF.Sigmoid, bias=b2_sb[:, 0:1])
            rw = tp.tile([32, 128], F32, tag="rw")
            nc.vector.tensor_mul(rw[:], scores[:], sel[:])
            sel8 = tp.tile([32, 128], U8, tag="sel8")
            nc.vector.tensor_copy(sel8[:], sel[:])

            nc.sync.dma_start(o_rw.rearrange("(f q) -> q f", q=32), rw[:])
            nc.sync.dma_start(o_sel.rearrange("(f q) -> q f", q=32), sel8[:])

    return nc


_NC_CACHE = {}


def _get_program():
    if "nc" not in _NC_CACHE:
        _NC_CACHE["nc"] = build_program()
    return _NC_CACHE["nc"]


def kernel(hidden_states, active_mask, W1, b1, W2, b2):
    hidden_states = np.asarray(hidden_states, dtype=np.float32)
    active_mask = np.asarray(active_mask)
    W1 = np.asarray(W1, dtype=np.float32)
    b1 = np.asarray(b1, dtype=np.float32)
    W2 = np.asarray(W2, dtype=np.float32)
    b2 = np.asarray(b2, dtype=np.float32)

    w1hi = W1.astype(np.float16)
    w1lo = (W1 - w1hi.astype(np.float32)).astype(np.float16)
    b1pk = np.ascontiguousarray(b1.reshape(HC, 128).T)
    w2pk = np.ascontiguousarray(W2[:, 0].reshape(HC, 128).T)
    b2rep = np.full((32, 1), b2[0], dtype=np.float32)

    in_maps = []
    for b in range(B):
        in_maps.append({
            "hs_t": np.ascontiguousarray(hidden_states[b].T),
            "w1hi": w1hi,
            "w1lo": w1lo,
            "b1pk": b1pk,
            "w2pk": w2pk,
            "b2rep": b2rep,
            "am_t": np.ascontiguousarray(
                active_mask[b].astype(np.uint8).reshape(128, 32).T),
        })

    nc = _get_program()
    res = run_bass_kernel_spmd(nc, in_maps, core_ids=list(range(B)))
    _NC_CACHE["last_results"] = res

    router_weights = np.stack([res.results[b]["o_rw"] for b in range(B)])
    selected_mask = np.stack([res.results[b]["o_sel"] for b in range(B)]).astype(bool)
    return router_weights, selected_mask



# revision 11
# speedup vs baseline: 2.3869x; 2.3869x over previous
"""Expert-choice MoE router on 8 Trainium2 NeuronCores.

Sharding: data-parallel over batch (B=8 rows -> 8 cores). Each core computes
its row's MLP router (Linear(4096,1024) -> exact GELU -> Linear(1024,1) ->
sigmoid) plus the per-row variable-k top-k selection.

v2 scheme (vs the 3-pass baseline): ONE fp16 pass on the PE array for the big
matmul (x and W1 pre-rounded to fp16 on host), which carries a deterministic
logit error of at most ~9e-4 (the dropped wlo*x + whi*xlo + wlo*xlo terms).
Selection exactness is restored by an on-device fixup of the ambiguous band:

  1. bisect an approximate threshold thr0 on the 1-pass logits,
  2. band = |logit - thr0| < 4e-3 (covers 2*err + bisect resolution;
     measured band size <= ~20 of 4096, capacity 64),
  3. compact band token ids with a matmul prefix-sum + one-hot gather,
  4. dma_gather (gpsimd, mlp ucode library) fetches those tokens' x rows in
     fp16 hi/lo parts, transposed into matmul layout,
  5. a 3-pass hi/lo fixup matmul on <=64 columns recomputes exact logits;
     pass 1 is bit-identical to the main pass, so delta = el3 - el1 applied
     to the stored logits replaces them with exact values,
  6. scatter delta back via a one-hot matmul, re-bisect, select.

Router weight VALUES use the approximate sigmoid scores (rel err ~2.5e-4);
only the selection bits need the exact comparison.
"""
import numpy as np

import concourse.bass as bass
import concourse.mybir as mybir
import concourse.tile as tile
from concourse import library_config
from concourse.bass_utils import run_bass_kernel_spmd
from concourse.library_overlay import lower_extended_insts

B, S, D, H = 8, 4096, 4096, 1024
KC = D // 128          # 32 contraction chunks
HC = H // 128          # 8 hidden chunks
TT = 512               # token tile (free dim of mm1)
NT = S // TT           # 8 token tiles
NSLOT = 64             # fixup capacity (band measured <= ~20)
NIDX = 128             # dma_gather num_idxs (hardware minimum granularity)
MARGIN = 4.0e-3        # band half-width
N_IT1 = 14             # bisect-1: res 16/2^14 ~ 1e-3
N_IT2 = 22             # bisect-2: res 16/2^22 ~ 4e-6 << min gap 5e-5
LOGIT_BOUND = 8.0

F32 = mybir.dt.float32
F16 = mybir.dt.float16
U8 = mybir.dt.uint8
I16 = mybir.dt.int16
I32 = mybir.dt.int32
AF = mybir.ActivationFunctionType
ALU = mybir.AluOpType


def _install_drain_split_patch():
    """The installed walrus build accepts fewer sync waits per instruction
    than bass/Tile emits; split multi-wait instructions into single-wait NOPs."""
    if getattr(tile.TileContext, "_drain_split_patched", False):
        return

    def split_multi_waits(nc, max_waits=1):
        ctr = 0
        for fn in nc.m.functions:
            for blk in fn.blocks:
                new = []
                changed = False
                for inst in blk.instructions:
                    si = inst.sync_info
                    waits = list(si.on_wait) if si is not None and si.on_wait else []
                    if len(waits) > max_waits:
                        for w in waits[:-max_waits]:
                            ctr += 1
                            new.append(mybir.InstNoOp(
                                name=f"WS-{ctr}",
                                engine=inst.engine,
                                sync_info=mybir.SyncInfo(on_wait=[w], on_update=[]),
                                bass_nofuse=True,
                            ))
                        si.on_wait = waits[-max_waits:]
                        changed = True
                    new.append(inst)
                if changed:
                    blk.instructions = new

    orig = tile.TileContext._drain_and_barrier

    def patched(self, tick_clock, wait_clock):
        orig(self, tick_clock, wait_clock)
        split_multi_waits(self.nc)

    tile.TileContext._drain_and_barrier = patched
    tile.TileContext._drain_split_patched = True


def _install_cost_model_dmasw_patch():
    """no_exec TimelineSim misses the hardware-implicit DMASW queue-semaphore
    increment (+16) that self-triggered (gen_mode=0) SWDGE DMAs perform at
    transfer completion; consumers that Tile gates on DMASW then deadlock the
    simulation. Append the increment to the instruction's timeline so the
    model matches hardware."""
    import concourse.cost_model as cm
    from concourse.cost_model_rust import SemUpdate as SemUpdateEvent

    if getattr(cm.InstructionCostModel, "_dmasw_patched", False):
        return

    _cache = {}

    def _dmasw_update(module, q):
        key = (id(module), q)
        if key not in _cache:
            found = None
            pref = f"DMASW{q}"
            for fn in module.functions:
                for blk in fn.blocks:
                    for inst in blk.instructions:
                        si = inst.sync_info
                        if si is None or not si.on_wait:
                            continue
                        for w in si.on_wait:
                            if w.ant_name and w.ant_name.startswith(pref):
                                found = mybir.SyncUpdate(
                                    sync_type="semaphore", id=w.id,
                                    ant_name=w.ant_name, update_value=16,
                                    update_mode="sem-add-imm")
                                break
                        if found is not None:
                            break
                    if found is not None:
                        break
                if found is not None:
                    break
            _cache[key] = found
        return _cache[key]

    orig = cm.InstructionCostModel.visit

    def visit(self, instruction, sim):
        timelines = orig(self, instruction, sim)
        if (isinstance(instruction, mybir.InstDMAGatherAnt)
                and instruction.gen_mode == 0):
            upd = _dmasw_update(sim.module.m, instruction.queue_num)
            if upd is not None:
                timelines = list(timelines)
                timelines[-1] = list(timelines[-1]) + [SemUpdateEvent(upd)]
        return timelines

    cm.InstructionCostModel.visit = visit
    cm.InstructionCostModel._dmasw_patched = True


def build_program():
    _install_drain_split_patch()
    _install_cost_model_dmasw_patch()
    nc = bass.Bass()

    hs16t = nc.dram_tensor("hs16t", [D, S], F16, kind="ExternalInput")
    hshilo = nc.dram_tensor("hshilo", [2 * S, D], F16, kind="ExternalInput")
    w1hi = nc.dram_tensor("w1hi", [D, H], F16, kind="ExternalInput")
    w1lo = nc.dram_tensor("w1lo", [D, H], F16, kind="ExternalInput")
    b1pk = nc.dram_tensor("b1pk", [128, HC], F32, kind="ExternalInput")
    w2pk = nc.dram_tensor("w2pk", [128, HC], F32, kind="ExternalInput")
    b2c = nc.dram_tensor("b2c", [128, 1], F32, kind="ExternalInput")
    am128 = nc.dram_tensor("am128", [128, NT * 4], U8, kind="ExternalInput")
    # tail constants
    iota_a = nc.dram_tensor("iota_a", [128, NSLOT], F32, kind="ExternalInput")
    iota_t = nc.dram_tensor("iota_t", [NSLOT, S], F32, kind="ExternalInput")
    l128 = nc.dram_tensor("l128", [128, 128], F16, kind="ExternalInput")
    l32 = nc.dram_tensor("l32", [32, 32], F32, kind="ExternalInput")
    ones_all = nc.dram_tensor("ones_all", [128, 128], F16, kind="ExternalInput")
    ones1 = nc.dram_tensor("ones1", [128, 1], F16, kind="ExternalInput")
    pp1 = nc.dram_tensor("pp1", [128, 1], F16, kind="ExternalInput")
    c128 = nc.dram_tensor("c128", [128, KC], F16, kind="ExternalInput")

    o_rw = nc.dram_tensor("o_rw", [S], F32, kind="ExternalOutput")
    o_sel = nc.dram_tensor("o_sel", [S], U8, kind="ExternalOutput")
    idx_scr = nc.dram_tensor("idx_scr", [2 * NIDX], I16, kind="Internal")

    hs_v = hs16t.rearrange("(k p) t -> k p t", p=128)
    w1hi_v = w1hi.rearrange("(k p) h -> p k h", p=128)
    w1lo_v = w1lo.rearrange("(k p) h -> p k h", p=128)

    with nc.semaphore("gsem") as gsem, nc.semaphore("psem") as psem, \
            tile.TileContext(nc) as tc:
        with tc.tile_pool(name="wres", bufs=1) as wres:
            # w1hi in 4 k-chunks so the first matmuls start after ~1/4 load
            w1hi_sb = []
            for i in range(4):
                wt = wres.tile([128, 8, H], F16, name=f"w1hi{i}")
                nc.sync.dma_start(wt[:], w1hi_v[:, 8 * i:8 * (i + 1), :])
                w1hi_sb.append(wt)
            w1lo_sb = wres.tile([128, KC, H], F16)
            nc.sync.dma_start(w1lo_sb[:], w1lo_v[:])
            b1_sb = wres.tile([128, HC], F32)
            nc.sync.dma_start(b1_sb[:], b1pk[:])
            w2_sb = wres.tile([128, HC], F32)
            nc.sync.dma_start(w2_sb[:], w2pk[:])
            b2_sb = wres.tile([128, 1], F32)
            nc.sync.dma_start(b2_sb[:], b2c[:])
            am_sb = wres.tile([128, NT * 4], U8)
            nc.sync.dma_start(am_sb[:], am128[:])
            logits = wres.tile([128, NT * 4], F32)

            nc.gpsimd.load_library(library_config.mlp)

            def w1hi_at(k, h):
                return w1hi_sb[k // 8][:, k % 8, h * 128:(h + 1) * 128]

            # ---------------- main 1-pass mm1 + gelu + mm2 ----------------
            with (
                tc.tile_pool(name="xin", bufs=6) as xin,
                tc.tile_pool(name="hact", bufs=2) as hp,
                tc.tile_pool(name="ps", bufs=8, space="PSUM") as ps,
            ):
                prev = None
                for T in range(NT):
                    psum = [ps.tile([128, TT], F32, tag="ps", name=f"psum{T}_{h}")
                            for h in range(HC)]
                    for k in range(KC):
                        xf = xin.tile([128, TT], F16, tag="xf")
                        nc.sync.dma_start(xf[:], hs_v[k, :, T * TT:(T + 1) * TT])
                        for h in range(HC):
                            nc.tensor.matmul(psum[h][:], w1hi_at(k, h), xf[:],
                                             start=(k == 0), stop=(k == KC - 1))
                    if prev is not None:
                        _emit_mm2(nc, ps, prev[0], w2_sb, logits, prev[1])
                    hacts = []
                    for h in range(HC):
                        ha = hp.tile([128, TT], F32, tag=f"ha{h}")
                        nc.scalar.activation(ha[:], psum[h][:], AF.Gelu,
                                             bias=b1_sb[:, h:h + 1])
                        hacts.append(ha)
                    prev = (hacts, T)
                _emit_mm2(nc, ps, prev[0], w2_sb, logits, prev[1])

            # ---------------- tail ----------------
            with (
                tc.tile_pool(name="tp", bufs=1) as tp,
                tc.tile_pool(name="tb", bufs=2) as tb,
                tc.tile_pool(name="gxp", bufs=1) as gxp,
                tc.tile_pool(name="ps2", bufs=1, space="PSUM") as ps2,
            ):
                iota_a_sb = tp.tile([128, NSLOT], F32)
                nc.sync.dma_start(iota_a_sb[:], iota_a[:])
                iota_t_sb = tp.tile([NSLOT, S], F32)
                nc.sync.dma_start(iota_t_sb[:], iota_t[:])
                l128_sb = tp.tile([128, 128], F16)
                nc.sync.dma_start(l128_sb[:], l128[:])
                l32_sb = tp.tile([32, 32], F32)
                nc.sync.dma_start(l32_sb[:], l32[:])
                onesa_sb = tp.tile([128, 128], F16)
                nc.sync.dma_start(onesa_sb[:], ones_all[:])
                ones1_sb = tp.tile([128, 1], F16)
                nc.sync.dma_start(ones1_sb[:], ones1[:])
                pp1_sb = tp.tile([128, 1], F16)
                nc.sync.dma_start(pp1_sb[:], pp1[:])
                c128_sb = tp.tile([128, KC], F16)
                nc.sync.dma_start(c128_sb[:], c128[:])

                def count_ge(src, scalar_col, tag):
                    """count over all 4096 elems of (src > scalar) -> [128,1] psum"""
                    ge = tb.tile([128, NT * 4], F32, tag="ge")
                    nc.vector.tensor_scalar(ge[:], src[:], scalar_col, None,
                                            op0=ALU.is_gt)
                    pcf = tb.tile([128, 1], F32, tag="pcf")
                    nc.vector.reduce_sum(pcf[:], ge[:], axis=mybir.AxisListType.X)
                    pc16 = tb.tile([128, 1], F16, tag="pc16")
                    nc.vector.tensor_copy(pc16[:], pcf[:])
                    cnt = ps2.tile([128, 1], F32, tag="cntq", name=f"cnt_{tag}")
                    nc.tensor.matmul(cnt[:], onesa_sb[:], pc16[:],
                                     start=True, stop=True)
                    return cnt

                # masked logits
                negbig = tp.tile([128, NT * 4], F32)
                nc.vector.memset(negbig[:], -1.0e30)
                ml = tp.tile([128, NT * 4], F32)
                nc.vector.select(ml[:], am_sb[:], logits[:], negbig[:])

                # num_active (replicated [128,1]) and k
                amf = tp.tile([128, NT * 4], F32)
                nc.vector.tensor_copy(amf[:], am_sb[:])
                pcf0 = tp.tile([128, 1], F32)
                nc.vector.reduce_sum(pcf0[:], amf[:], axis=mybir.AxisListType.X)
                pc160 = tp.tile([128, 1], F16)
                nc.vector.tensor_copy(pc160[:], pcf0[:])
                na_ps = ps2.tile([128, 1], F32, tag="cntq", name="na_ps")
                nc.tensor.matmul(na_ps[:], onesa_sb[:], pc160[:], start=True, stop=True)
                na = tp.tile([128, 1], F32)
                nc.vector.tensor_copy(na[:], na_ps[:])
                # k = min(max(rne(0.5*na - 0.25), 1), na)  (== clamped floor)
                k0f = tp.tile([128, 1], F32)
                nc.vector.tensor_scalar(k0f[:], na[:], 0.5, -0.25,
                                        op0=ALU.mult, op1=ALU.add)
                k0i = tp.tile([128, 1], I32)
                nc.vector.tensor_copy(k0i[:], k0f[:])
                kf = tp.tile([128, 1], F32)
                nc.vector.tensor_copy(kf[:], k0i[:])
                nc.vector.tensor_scalar_max(kf[:], kf[:], 1.0)
                krep = tp.tile([128, 1], F32)
                nc.vector.tensor_tensor(krep[:], kf[:], na[:], op=ALU.min)

                def bisect(src, lo, hi, n_iter, tag):
                    for it in range(n_iter):
                        mid = tb.tile([128, 1], F32, tag=f"mid{tag}")
                        nc.vector.tensor_tensor(mid[:], lo[:], hi[:], op=ALU.add)
                        nc.vector.tensor_scalar_mul(mid[:], mid[:], 0.5)
                        cnt = count_ge(src, mid[:], f"{tag}_{it}")
                        gek = tb.tile([128, 1], U8, tag=f"gek{tag}")
                        nc.vector.tensor_tensor(gek[:], cnt[:], krep[:], op=ALU.is_ge)
                        nlo = tb.tile([128, 1], F32, tag=f"lo{tag}")
                        nc.vector.select(nlo[:], gek[:], mid[:], lo[:])
                        nhi = tb.tile([128, 1], F32, tag=f"hi{tag}")
                        nc.vector.select(nhi[:], gek[:], hi[:], mid[:])
                        lo, hi = nlo, nhi
                    return lo, hi

                lo0 = tp.tile([128, 1], F32)
                nc.vector.memset(lo0[:], -LOGIT_BOUND)
                hi0 = tp.tile([128, 1], F32)
                nc.vector.memset(hi0[:], LOGIT_BOUND)
                thr0, _ = bisect(ml, lo0, hi0, N_IT1, "b1")

                # band mask amb = (ml > thr0 - m) & (ml < thr0 + m)
                lob = tp.tile([128, 1], F32)
                nc.vector.tensor_scalar(lob[:], thr0[:], -MARGIN, None, op0=ALU.add)
                hib = tp.tile([128, 1], F32)
                nc.vector.tensor_scalar(hib[:], thr0[:], MARGIN, None, op0=ALU.add)
                ge1 = tp.tile([128, NT * 4], F32)
                nc.vector.tensor_scalar(ge1[:], ml[:], lob[:], None, op0=ALU.is_gt)
                lt1 = tp.tile([128, NT * 4], F32)
                nc.vector.tensor_scalar(lt1[:], ml[:], hib[:], None, op0=ALU.is_lt)
                amb = tp.tile([128, NT * 4], F32)
                nc.vector.tensor_mul(amb[:], ge1[:], lt1[:])
                amb16 = tp.tile([128, NT * 4], F16)
                nc.vector.tensor_copy(amb16[:], amb[:])

                # prefix-sum slot assignment: slot[t] = #ambiguous tokens with id < t
                wp_ps = ps2.tile([128, KC], F32, tag="pfa", name="wp_ps")
                nc.tensor.matmul(wp_ps[:], l128_sb[:], amb16[:], start=True, stop=True)
                cs_ps = ps2.tile([1, KC], F32, tag="pfb", name="cs_ps")
                nc.tensor.matmul(cs_ps[:], ones1_sb[:], amb16[:], start=True, stop=True)
                cs32 = tp.tile([32, 32], F32)
                nc.vector.memset(cs32[:], 0.0)
                nc.vector.tensor_copy(cs32[0:1, :], cs_ps[:])
                csT = tp.tile([32, 32], F32)
                nc.vector.transpose(csT[:], cs32[:])
                xrep = tp.tile([32, 128], F32)
                nc.vector.tensor_copy(xrep[:], csT[:, 0:1].broadcast_to([32, 128]))
                ccs_ps = ps2.tile([128, KC], F32, tag="pfb", name="ccs_ps")
                nc.tensor.matmul(ccs_ps[:], xrep[:], l32_sb[:], start=True, stop=True)
                wp_sb = tp.tile([128, KC], F32)
                nc.vector.tensor_copy(wp_sb[:], wp_ps[:])
                prefix = tp.tile([128, KC], F32)
                nc.vector.tensor_tensor(prefix[:], wp_sb[:], ccs_ps[:], op=ALU.add)
                # slotm = amb ? prefix : -1  ==  (prefix+1)*amb - 1
                nc.vector.tensor_scalar(prefix[:], prefix[:], 1.0, None, op0=ALU.add)
                nc.vector.tensor_mul(prefix[:], prefix[:], amb[:])
                nc.vector.tensor_scalar(prefix[:], prefix[:], -1.0, None, op0=ALU.add)

                # one-hot G[p, c, a] = (slot[p, c] == a), token t = c*128+p
                G = tp.tile([128, KC, NSLOT], F16)
                for c in range(KC):
                    nc.vector.tensor_scalar(G[:, c, :], iota_a_sb[:],
                                            prefix[:, c:c + 1], None, op0=ALU.is_equal)
                # idx[a] = token id of slot a (or -1): split exact-fp16 parts
                idxa_ps = ps2.tile([NSLOT, 1], F32, tag="pfa", name="idxa_ps")
                idxb_ps = ps2.tile([NSLOT, 1], F32, tag="pfb", name="idxb_ps")
                for c in range(KC):
                    nc.tensor.matmul(idxa_ps[:], G[:, c, :], pp1_sb[:],
                                     start=(c == 0), stop=(c == KC - 1))
                    nc.tensor.matmul(idxb_ps[:], G[:, c, :], c128_sb[:, c:c + 1],
                                     start=(c == 0), stop=(c == KC - 1))
                idxa_sb = tp.tile([NSLOT, 1], F32)
                nc.vector.tensor_copy(idxa_sb[:], idxa_ps[:])
                idx = tp.tile([NSLOT, 1], F32)
                nc.vector.tensor_tensor(idx[:], idxa_sb[:], idxb_ps[:], op=ALU.add)
                nc.vector.tensor_scalar(idx[:], idx[:], -1.0, None, op0=ALU.add)
                gidx = tp.tile([NSLOT, 1], F32)
                nc.vector.tensor_scalar_max(gidx[:], idx[:], 0.0)
                gidx16 = tp.tile([NSLOT, 1], I16)
                nc.vector.tensor_copy(gidx16[:], gidx[:])
                glo = tp.tile([NSLOT, 1], F32)
                nc.vector.tensor_scalar(glo[:], gidx[:], float(S), None, op0=ALU.add)
                glo16 = tp.tile([NSLOT, 1], I16)
                nc.vector.tensor_copy(glo16[:], glo[:])
                z16 = tp.tile([NSLOT, 1], I16)
                nc.vector.memset(z16[:], 0)
                zlo16 = tp.tile([NSLOT, 1], I16)
                nc.vector.memset(zlo16[:], S)

                # combined hi+lo idx list -> DRAM -> wrapped [16, 16] -> [128, 16]
                nc.sync.dma_start(idx_scr[0:NSLOT], gidx16[:])
                nc.sync.dma_start(idx_scr[NSLOT:NIDX], z16[:])
                nc.sync.dma_start(idx_scr[NIDX:NIDX + NSLOT], glo16[:])
                nc.sync.dma_start(idx_scr[NIDX + NSLOT:2 * NIDX], zlo16[:])
                irep = tp.tile([128, 2 * NIDX // 16], I16)
                idx_wrap = idx_scr.rearrange("(f q) -> q f", q=16)
                for r in range(8):
                    nc.sync.dma_start(irep[16 * r:16 * (r + 1), :], idx_wrap)

                gxhl = gxp.tile([128, KC, 2 * NIDX], F16)
                nc.gpsimd.dma_gather(gxhl[:], hshilo[:, :], irep[:],
                                     num_idxs=2 * NIDX, num_idxs_reg=2 * NIDX,
                                     elem_size=D,
                                     transpose=True).then_inc(gsem, 16)
                nc.gpsimd.wait_ge(gsem, 16)
                tc.no_sync_barrier()
                nc.tensor.wait_ge(gsem, 16)
                tc.no_sync_barrier()

                # fixup: pass1 (bit-identical to main) + hi/lo corrections
                psum1 = ps2.tile([128, HC, NSLOT], F32, tag="big1", name="psum1")
                for k in range(KC):
                    for h in range(HC):
                        nc.tensor.matmul(psum1[:, h, :], w1hi_at(k, h),
                                         gxhl[:, k, 0:NSLOT],
                                         start=(k == 0), stop=(k == KC - 1))
                psumc = ps2.tile([128, HC, NSLOT], F32, tag="big2", name="psumc")
                for k in range(KC):
                    for h in range(HC):
                        nc.tensor.matmul(psumc[:, h, :], w1hi_at(k, h),
                                         gxhl[:, k, NIDX:NIDX + NSLOT],
                                         start=(k == 0), stop=False)
                for k in range(KC):
                    for h in range(HC):
                        nc.tensor.matmul(psumc[:, h, :],
                                         w1lo_sb[:, k, h * 128:(h + 1) * 128],
                                         gxhl[:, k, 0:NSLOT],
                                         start=False, stop=(k == KC - 1))
                p1_sb = tp.tile([128, HC, NSLOT], F32)
                nc.vector.tensor_copy(p1_sb[:], psum1[:])
                pre3 = tp.tile([128, HC, NSLOT], F32)
                nc.vector.tensor_tensor(pre3[:], p1_sb[:], psumc[:], op=ALU.add)
                hact1 = tp.tile([128, HC, NSLOT], F32)
                hact3 = tp.tile([128, HC, NSLOT], F32)
                for h in range(HC):
                    nc.scalar.activation(hact1[:, h, :], psum1[:, h, :], AF.Gelu,
                                         bias=b1_sb[:, h:h + 1])
                    nc.scalar.activation(hact3[:, h, :], pre3[:, h, :], AF.Gelu,
                                         bias=b1_sb[:, h:h + 1])
                el1_ps = ps2.tile([NSLOT, 1], F32, tag="pfa", name="el1_ps")
                el3_ps = ps2.tile([NSLOT, 1], F32, tag="pfb", name="el3_ps")
                for h in range(HC):
                    nc.tensor.matmul(el1_ps[:], hact1[:, h, :], w2_sb[:, h:h + 1],
                                     start=(h == 0), stop=(h == HC - 1))
                for h in range(HC):
                    nc.tensor.matmul(el3_ps[:], hact3[:, h, :], w2_sb[:, h:h + 1],
                                     start=(h == 0), stop=(h == HC - 1))
                el1_sb = tp.tile([NSLOT, 1], F32)
                nc.vector.tensor_copy(el1_sb[:], el1_ps[:])
                dlt = tp.tile([NSLOT, 1], F32)
                nc.vector.tensor_sub(dlt[:], el3_ps[:], el1_sb[:])
                dlt16 = tp.tile([NSLOT, 1], F16)
                nc.vector.tensor_copy(dlt16[:], dlt[:])

                # scatter: M_T[a, t] = (idx[a] == t); delta = M_T^T @ dlt
                mt = tp.tile([NSLOT, S], F16)
                nc.vector.tensor_scalar(mt[:], iota_t_sb[:], idx[:], None,
                                        op0=ALU.is_equal)
                dps = ps2.tile([128, KC], F32, tag="dpst", name="dps")
                for c in range(KC):
                    nc.tensor.matmul(dps[:, c:c + 1], mt[:, c * 128:(c + 1) * 128],
                                     dlt16[:], start=True, stop=True)
                merged = tp.tile([128, NT * 4], F32)
                nc.vector.tensor_tensor(merged[:], ml[:], dps[:], op=ALU.add)

                lo2 = tp.tile([128, 1], F32)
                nc.vector.tensor_scalar(lo2[:], thr0[:], -MARGIN, None, op0=ALU.add)
                hi2 = tp.tile([128, 1], F32)
                nc.vector.memset(hi2[:], LOGIT_BOUND)
                thr2, _ = bisect(merged, lo2, hi2, N_IT2, "b2")

                sel = tp.tile([128, NT * 4], F32)
                nc.vector.tensor_scalar(sel[:], merged[:], thr2[:], None,
                                        op0=ALU.is_gt)
                scores = tp.tile([128, NT * 4], F32)
                nc.scalar.activation(scores[:], logits[:], AF.Sigmoid,
                                     bias=b2_sb[:, 0:1])
                rw = tp.tile([128, NT * 4], F32)
                nc.vector.tensor_mul(rw[:], scores[:], sel[:])
                sel8 = tp.tile([128, NT * 4], U8)
                nc.vector.tensor_copy(sel8[:], sel[:])

                nc.sync.dma_start(o_rw.rearrange("(c p) -> p c", p=128), rw[:])
                nc.sync.dma_start(o_sel.rearrange("(c p) -> p c", p=128), sel8[:])

    lower_extended_insts(nc)
    return nc


def _emit_mm2(nc, ps, hacts, w2_sb, logits, T):
    ps2t = ps.tile([128, 4], F32, tag="ps", name=f"ps2_{T}")
    for c in range(4):
        for h in range(HC):
            nc.tensor.matmul(ps2t[:, c:c + 1],
                             hacts[h][:, c * 128:(c + 1) * 128],
                             w2_sb[:, h:h + 1],
                             start=(h == 0), stop=(h == HC - 1))
    nc.vector.tensor_copy(logits[:, T * 4:(T + 1) * 4], ps2t[:])


_NC_CACHE = {}


def _get_program():
    if "nc" not in _NC_CACHE:
        _NC_CACHE["nc"] = build_program()
    return _NC_CACHE["nc"]


def _host_constants():
    iota_a = np.broadcast_to(np.arange(NSLOT, dtype=np.float32), (128, NSLOT))
    iota_t = np.broadcast_to(np.arange(S, dtype=np.float32), (NSLOT, S))
    l128 = np.triu(np.ones((128, 128), np.float16), 1)
    l32 = np.triu(np.ones((32, 32), np.float32), 1)
    ones_all = np.ones((128, 128), np.float16)
    ones1 = np.ones((128, 1), np.float16)
    pp1 = (np.arange(128, dtype=np.float16) + 1).reshape(128, 1)
    c128 = np.broadcast_to(np.arange(KC, dtype=np.float32) * 128,
                           (128, KC)).astype(np.float16)
    return {
        "iota_a": np.ascontiguousarray(iota_a),
        "iota_t": np.ascontiguousarray(iota_t),
        "l128": l128, "l32": l32,
        "ones_all": ones_all, "ones1": ones1,
        "pp1": pp1, "c128": np.ascontiguousarray(c128),
    }


def kernel(hidden_states, active_mask, W1, b1, W2, b2):
    hidden_states = np.asarray(hidden_states, dtype=np.float32)
    active_mask = np.asarray(active_mask)
    W1 = np.asarray(W1, dtype=np.float32)
    b1 = np.asarray(b1, dtype=np.float32)
    W2 = np.asarray(W2, dtype=np.float32)
    b2 = np.asarray(b2, dtype=np.float32)

    w1hi = W1.astype(np.float16)
    w1lo = (W1 - w1hi.astype(np.float32)).astype(np.float16)
    b1pk = np.ascontiguousarray(b1.reshape(HC, 128).T)
    w2pk = np.ascontiguousarray(W2[:, 0].reshape(HC, 128).T)
    b2c = np.full((128, 1), b2[0], dtype=np.float32)
    consts = _host_constants()

    in_maps = []
    for b in range(B):
        hs16 = hidden_states[b].astype(np.float16)
        hs16lo = (hidden_states[b] - hs16.astype(np.float32)).astype(np.float16)
        in_maps.append({
            "hs16t": np.ascontiguousarray(hs16.T),
            "hshilo": np.concatenate([hs16, hs16lo], axis=0),
            "w1hi": w1hi,
            "w1lo": w1lo,
            "b1pk": b1pk,
            "w2pk": w2pk,
            "b2c": b2c,
            "am128": np.ascontiguousarray(
                active_mask[b].astype(np.uint8).reshape(NT * 4, 128).T),
            **consts,
        })

    nc = _get_program()
    res = run_bass_kernel_spmd(nc, in_maps, core_ids=list(range(B)))
    _NC_CACHE["last_results"] = res

    router_weights = np.stack([res.results[b]["o_rw"] for b in range(B)])
    selected_mask = np.stack([res.results[b]["o_sel"] for b in range(B)]).astype(bool)
    return router_weights, selected_mask
